# revision 11
# baseline (speedup 1.0000x reference)
"""MMD loss kernel for Trainium2, SPMD across 8 NeuronCores.

Math: loss = (1/B^2) * sum_{ij} s_i s_j K_ij over the [2B, 2B] Gaussian
kernel-sum matrix, s = [+1]*B ++ [-1]*B.  K_ij = sum_{k=0..4} exp(-l2_ij / (bw*2^k))
with bw = mean off-diagonal l2 / 4 (computed on host via the algebraic identity
sum(l2) = 2N*sum(sq) - 2*||sum x||^2).

Device strategy per core (SPMD, identical program; per-core data sliced on host):
  - 16x16 grid of 512-wide blocks over the symmetric 8192x8192 matrix.
    Core c owns block-rows {2c, 2c+1}. 17 blocks per core: (r0, d=0..7),
    (r1, d=0..7) and ONE wrap-diagonal d=8 block at weight 2 (cores 0-3 take
    the even row's pair, cores 4-7 the odd row's; slots 9/10 hold its data so
    the program is identical across cores). d=0 weight 1, d=1..7 weight 2.
  - PSUM accumulates P = x_i.x_j - (sq_i+sq_j)/2 = -l2/2 via 4 fp8e4m3
    DoubleRow matmuls (kd pairs, 2x PE rate) plus one K=2 fp32r matmul adding
    the exact -(sq_i+sq_j)/2 (strip-packed: 4 concurrent row-group MMs).
    The lhsT is the same fp8 v tile as the rhs (no separate scaled copy;
    the former 2x factor is folded into the exp scales).
  - Elementwise (5 kernel levels, sums only): ACT: t4 = exp(8c*P) and
    t1 = exp(2c*P) with fused row-sum accums (sum t4, sum t1). DVE:
    t8 = t4*t4 via plain TENSOR_TENSOR (the only op with 2x packed-bf16
    uops; every accum-bearing DVE op runs at 1x), then ONE bn_stats+bn_aggr
    pass over t8 yields mean/var -> sum(t8) AND sum(t16) without ever
    materializing t16. sum(t2): TT square + Pool-engine tensor_reduce if
    POOL_T2, else a 1x STT with accum on DVE.
  - Last block is computed in 4 column-quarters (MMs + elementwise) so the
    post-matmul drain tail is ~3us instead of ~7us.
  - Host reduces the [128, 100] per-core level-sums with block weights/signs.
"""

import sys

sys.path.insert(0, "/opt/trn_rl_repo")

import numpy as np
import ml_dtypes

import concourse.mybir as mybir
import concourse.tile as tile
from concourse import bacc
from concourse.bass_utils import run_bass_kernel_spmd

B = 4096
D = 1024
N = 2 * B
NB = 16          # block grid (512-wide)
BS = 512
KD = 8           # feature k-tiles of 128
G = 4            # kd pairs (DoubleRow groups)
NCORES = 8
NVSLOT = 11      # v slots: 0..8 consecutive cols, 9/10 the wrap-pair block
NBLK = 17
NSLOT = 16 * 6 + 24   # [s1, s2_dve, s4, mean8, var8, s2_act]; last block quartered

FP8 = mybir.dt.float8e4
BF16 = mybir.dt.bfloat16
F32 = mybir.dt.float32
F32R = mybir.dt.float32r
NP_FP8 = ml_dtypes.float8_e4m3

# program block list: (lhsT slot, rhs slot)
BLOCKS = [(0, d) for d in range(8)] + [(1, 1 + d) for d in range(8)] + [(9, 10)]
UACOL = {0: 0, 1: 1, 9: 2}   # lhsT slot -> aug column holding -sq/2

_prog_cache = {}


def build_program():
    if "nc" in _prog_cache:
        return _prog_cache["nc"]
    nc = bacc.Bacc("TRN2", target_bir_lowering=False, debug=False, num_devices=NCORES)
    v_d = nc.dram_tensor("v", [NVSLOT, 128, KD, BS], FP8, kind="ExternalInput").ap()
    aug_d = nc.dram_tensor("aug", [8, 14, BS], F32R, kind="ExternalInput").ap()
    sc_d = nc.dram_tensor("sc", [128, 3], F32, kind="ExternalInput").ap()
    out_d = nc.dram_tensor("out", [128, NSLOT], F32, kind="ExternalOutput").ap()

    MULT = mybir.AluOpType.mult
    ADD = mybir.AluOpType.add
    EXP = mybir.ActivationFunctionType.Exp
    SQUARE = mybir.ActivationFunctionType.Square
    DR = mybir.MatmulPerfMode.DoubleRow

    with tile.TileContext(nc) as tc:
        with (
            tc.tile_pool(name="vstat", bufs=1) as vpool,
            tc.tile_pool(name="augp", bufs=1) as augpool,
            tc.tile_pool(name="cst", bufs=1) as cstpool,
            tc.tile_pool(name="ot", bufs=1) as opool,
            tc.tile_pool(name="texp", bufs=3) as tpool,
            tc.tile_pool(name="wsq", bufs=2) as wpool,
            tc.tile_pool(name="ps", bufs=2, space="PSUM") as pspool,
        ):
            v_sb = vpool.tile([128, NVSLOT, KD, BS], FP8)
            aug_sb = augpool.tile([128, 14, BS], F32R)
            sc_sb = cstpool.tile([128, 3], F32)
            out_sb = opool.tile([128, NSLOT], F32)

            # DMA order == consumption order. Large whole-slot transfers
            # amortize the ~600ns sync-engine DMA_DIRECT2D issue cost that
            # throttled the old per-kd interleave to ~200 GB/s; v0/v1 are
            # chunked so the first blocks' matmuls can start early.
            nc.sync.dma_start(out=v_sb[:, 0, 0:2, :], in_=v_d[0, :, 0:2, :])
            nc.sync.dma_start(out=sc_sb[:], in_=sc_d[:])
            # aug strips: partitions 32g+0 get (ua=-sq/2, va=+1) rows,
            # partitions 32g+1 get (ua=-1, va=sq/2) rows. Issued early: the
            # aug matmul ends every psum group, so a late aug DMA stalls the
            # whole ACT/DVE chain behind block 0.
            nc.sync.dma_start(out=aug_sb[0:128:32, :, :], in_=aug_d[0:4, :, :])
            nc.sync.dma_start(out=aug_sb[1:128:32, :, :], in_=aug_d[4:8, :, :])
            for g in range(1, G):
                nc.sync.dma_start(
                    out=v_sb[:, 0, 2 * g:2 * g + 2, :],
                    in_=v_d[0, :, 2 * g:2 * g + 2, :],
                )
            for h in range(2):
                nc.sync.dma_start(
                    out=v_sb[:, 1, 4 * h:4 * h + 4, :],
                    in_=v_d[1, :, 4 * h:4 * h + 4, :],
                )
            for s in range(2, NVSLOT):
                nc.sync.dma_start(out=v_sb[:, s], in_=v_d[s])

            # PE warm-up: ~5us of junk matmuls on a memset tile, issued with
            # no DMA dependency so they run during the first chunk's ~5us
            # DMA latency. Keeps the HAM activity window busy so the real
            # matmul stream starts at 2.4 GHz instead of 1.2.
            wtile = cstpool.tile([128, 8], BF16)
            nc.gpsimd.memset(wtile[:], 0.25)
            ps_w = pspool.tile([128, 4, BS], F32, name="ps_warm", tag="ps")
            for _ in range(56):
                nc.tensor.matmul(
                    ps_w[0:8, 0, 0:8],
                    lhsT=wtile[:, :],
                    rhs=wtile[:, :],
                    start=True,
                    stop=True,
                )

            def emit_mms(ps, lb, rb, cs, cp):
                for g in range(G):
                    for it in range(4):
                        nc.tensor.matmul(
                            ps[:, it, cp],
                            lhsT=v_sb[:, lb, 2 * g:2 * g + 2, it * 128:(it + 1) * 128],
                            rhs=v_sb[:, rb, 2 * g:2 * g + 2, cs],
                            start=(g == 0),
                            stop=False,
                            perf_mode=DR,
                        )
                for it in range(4):
                    nc.tensor.matmul(
                        ps[:, it, cp],
                        lhsT=aug_sb[32 * it:32 * it + 2, UACOL[lb], it * 128:(it + 1) * 128],
                        rhs=aug_sb[32 * it:32 * it + 2, 3 + rb, cs],
                        start=False,
                        stop=True,
                        tile_position=(32 * it, 0),
                    )

            def tt_mult(out, in_):
                """Plain TENSOR_TENSOR square on DVE: the only op family
                with 2x packed-bf16 uops (accum-bearing variants are 1x)."""
                ve = nc.vector
                ve.add_instruction(
                    mybir.InstTensorTensor(
                        name=nc.get_next_instruction_name(),
                        op=MULT,
                        ins=[ve.lower_ap(in_), ve.lower_ap(in_)],
                        outs=[ve.lower_ap(out)],
                    )
                )

            def emit_elem(ps, t4, t1, t8, t2, bn6, cs, cp, sbase):
                # slots: [s1, s2_dve, s4, mean(t8), var(t8), s2_act]
                oc = [out_sb[:, sbase + k:sbase + k + 1] for k in range(6)]
                nc.scalar.activation(
                    t4[:, :, cs], ps[:, :, cp], EXP,
                    scale=sc_sb[:, 2:3], accum_out=oc[2],
                )
                nc.scalar.activation(
                    t1[:, :, cs], ps[:, :, cp], EXP,
                    scale=sc_sb[:, 0:1], accum_out=oc[0],
                )
                tt_mult(t8[:, :, cs], t4[:, :, cs])
                # bn_stats is capped at 512 free elems -> one call per bank;
                # one bn_aggr then yields mean/var -> sum(t8) AND sum(t16)
                # without materializing t16 (accum-bearing DVE ops are 1x,
                # so halving the summed elements is the whole game).
                for g in range(4):
                    nc.vector.bn_stats(bn6[:, g, :], t8[:, g, cs])
                nc.vector.bn_aggr(out_sb[:, sbase + 3:sbase + 5], bn6[:])
                # sum(t2): bank 0 on ACT (Square w/ accum), banks 1-3 on DVE
                # (STT w/ accum) - balances the two engines at ~5.5us/block
                nc.scalar.activation(
                    t2[:, 0, cs], t1[:, 0, cs], SQUARE, accum_out=oc[5],
                )
                nc.vector.scalar_tensor_tensor(
                    out=t2[:, 1:4, cs], in0=t1[:, 1:4, cs], scalar=1.0,
                    in1=t1[:, 1:4, cs], op0=MULT, op1=MULT, accum_out=oc[1],
                )

            full = slice(0, BS)
            for b, (lb, rb) in enumerate(BLOCKS):
                last = b == NBLK - 1
                t4 = tpool.tile([128, 4, BS], BF16, name=f"t4_{b}", tag="t4")
                t1 = tpool.tile([128, 4, BS], BF16, name=f"t1_{b}", tag="t1")
                t8 = wpool.tile([128, 4, BS], BF16, name=f"t8_{b}", tag="t8")
                t2 = wpool.tile([128, 4, BS], BF16, name=f"t2_{b}", tag="t2")
                bn6 = wpool.tile([128, 4, 6], F32, name=f"bn_{b}", tag="bn")
                if not last:
                    ps = pspool.tile([128, 4, BS], F32, name=f"ps_{b}", tag="ps")
                    emit_mms(ps, lb, rb, full, full)
                    emit_elem(ps, t4, t1, t8, t2, bn6, full, full, 6 * b)
                else:
                    # halves ping-pong between the two psum buffers so the
                    # second half's matmuls overlap the first's elementwise
                    for q in range(2):
                        qs = slice(q * 256, (q + 1) * 256)
                        qps = slice(0, 256)
                        ps = pspool.tile(
                            [128, 4, BS], F32, name=f"ps_{b}_{q}", tag="ps"
                        )
                        emit_mms(ps, lb, rb, qs, qps)
                        emit_elem(ps, t4, t1, t8, t2, bn6, qs, qps, 96 + 6 * q)
                if b == 14:
                    nc.sync.dma_start(out=out_d[:, 0:90], in_=out_sb[:, 0:90])
            nc.sync.dma_start(out=out_d[:, 90:NSLOT], in_=out_sb[:, 90:NSLOT])
    nc.compile()
    _prog_cache["nc"] = nc
    return nc


def core_slots(c):
    """Global 512-col block indices held by v slots 0..10 on core c."""
    a0 = 2 * c
    slots = [(a0 + s) % NB for s in range(9)]
    if c < 4:
        slots += [a0 % NB, (a0 + 8) % NB]
    else:
        slots += [(a0 + 1) % NB, (a0 + 9) % NB]
    return slots


def prepare_inputs(source: np.ndarray, target: np.ndarray):
    """Host-side shard prep. Returns in_maps for the 8 cores."""
    total = np.concatenate([source, target], axis=0).astype(np.float32)  # [N, D]
    t64 = total.astype(np.float64)
    sq64 = np.einsum("nd,nd->n", t64, t64)
    S1 = sq64.sum()
    vsum = t64.sum(axis=0)
    sum_l2 = 2.0 * N * S1 - 2.0 * (vsum @ vsum)
    bandwidth = sum_l2 / (N * N - N)
    bandwidth = bandwidth / (2.0 ** (5 // 2))  # KERNEL_MUL ** (KERNEL_NUM // 2)
    c4 = np.float64(1.0) / (16.0 * bandwidth)

    sq32 = sq64.astype(np.float32)
    Tt = np.ascontiguousarray(total.T)  # [D, N] f32
    v_all = Tt.astype(NP_FP8).reshape(KD, 128, N)

    # psum holds -l2/2, so all exp scales are doubled vs the -l2 formulation
    sc_np = np.empty((128, 3), dtype=np.float32)
    sc_np[:, 0] = np.float32(2.0 * c4)
    sc_np[:, 1] = np.float32(4.0 * c4)
    sc_np[:, 2] = np.float32(8.0 * c4)

    in_maps = []
    for c in range(NCORES):
        slots = core_slots(c)
        v_np = np.empty((NVSLOT, 128, KD, BS), dtype=NP_FP8)
        aug_np = np.zeros((8, 14, BS), dtype=np.float32)
        for s, gcol in enumerate(slots):
            cols = slice(gcol * BS, (gcol + 1) * BS)
            v_np[s] = v_all[:, :, cols].transpose(1, 0, 2)
        for g in range(4):
            for li, sl in enumerate([slots[0], slots[1], slots[9]]):
                aug_np[g, li] = -0.5 * sq32[sl * BS:(sl + 1) * BS]
            aug_np[g, 3:14] = 1.0
            aug_np[4 + g, 0:3] = -1.0
            for s, gcol in enumerate(slots):
                aug_np[4 + g, 3 + s] = 0.5 * sq32[gcol * BS:(gcol + 1) * BS]
        in_maps.append({"v": v_np, "aug": aug_np, "sc": sc_np})
    return in_maps


def reduce_outputs(outs):
    """outs: list of [128, NSLOT] f32 per core -> loss (np.float32 scalar)."""
    S = 0.0
    for c in range(NCORES):
        o = outs[c].astype(np.float64)  # [128, NSLOT]
        cols = o.sum(axis=0)  # [NSLOT]
        slots = core_slots(c)
        a0 = 2 * c
        for b, (lb, rb) in enumerate(BLOCKS):
            if b < 16:
                bases, n = [6 * b], 2048.0
            else:
                bases, n = [96 + 6 * q for q in range(2)], 1024.0
            bsum = 0.0
            for base in bases:
                s124 = cols[base + 0] + cols[base + 1] + cols[base + 2] + cols[base + 5]
                m8 = o[:, base + 3]
                v8 = o[:, base + 4]
                s8 = (m8 * n).sum()
                s16 = ((v8 + m8 * m8) * n).sum()
                bsum += s124 + s8 + s16
            grow = slots[lb]
            gcol = slots[rb]
            w = 1.0 if (b == 0 or b == 8) else 2.0
            sr = 1.0 if grow < NB // 2 else -1.0
            sg = 1.0 if gcol < NB // 2 else -1.0
            S += w * sr * sg * bsum
    return np.float32(S / (float(B) * float(B)))


def kernel(source: np.ndarray, target: np.ndarray) -> np.ndarray:
    nc = build_program()
    in_maps = prepare_inputs(source, target)
    res = run_bass_kernel_spmd(nc, in_maps, list(range(NCORES)))
    outs = [res.results[c]["out"] for c in range(NCORES)]
    return np.asarray(reduce_outputs(outs), dtype=np.float32)
